# revision 11
# baseline (speedup 1.0000x reference)
"""Trainium2 Bass kernel for nn_ContinuousCoprimality.

Per batch row r of two [4096, 16384] fp32 tensors computes
    c_i  = #{x_i[r, :] > 0}
    c_j  = #{x_j[r, :] > 0}
    c_ij = #{(x_i + x_j)[r, :] > 0}
on 8 NeuronCores (rows sharded 512/core); the tiny binary-entropy / E /
threshold tail runs on host in float32, mirroring the reference jnp
arithmetic exactly.

Device-side layout per core: the [512, 16384] shard is a flat buffer viewed
as 16 "megas" of [128 partitions x 4096 fp32] (each partition = one quarter
row, so a mega holds 32 whole rows; DMA is perfectly contiguous).
Per mega:
  DVE:  scr_s = x_i + x_j (fp32, exact);  q_i = (x_i > 0), q_s = (scr_s > 0)
        as bf16 0/1
  ACT:  sg = Sign(x_j); sg = Relu(sg)  -> strict (x_j > 0) indicator, bf16
  PE :  24 matmuls vs a constant block-ones lhsT [128, 32] reduce the
        partition dim (4 quarter-rows -> row) into PSUM [32, 512],
        accumulating the 8 free-dim slices of each quantized tile
  DVE:  tensor_reduce over PSUM free dim -> counts[32, col]
Counts [32, 48] are DMA'd out once; host combines and finishes.

Only production-proven instruction forms are used (plain TT/TS/Activation/
Matmult/TensorReduce) — the fused accum_out variants of TensorScalar /
Activation hit "Too many sync wait commands" in this walrus codegen.
"""

import os as _os

import numpy as np

B, F = 4096, 16384
N_CORES = 8
R = B // N_CORES        # 512 rows per core
P = 128                 # SBUF partitions
W = 4096                # fp32 per partition per mega (quarter row)
QUART = F // W          # 4 partitions per row
ROWS_PER_MEGA = P // QUART  # 32
M = (R * F) // (P * W)  # 16 megas per core
if _os.environ.get("KERNEL_M_OVERRIDE"):
    M = int(_os.environ["KERNEL_M_OVERRIDE"])
LOOPS = int(_os.environ.get("KERNEL_LOOPS", "1"))
NSLICE = W // 512       # matmul free-dim slices per tile

_CACHE = {}
LAST_RESULT = None


def _ones_block_np():
    import ml_dtypes
    w = np.zeros((P, ROWS_PER_MEGA), dtype=np.float32)
    for k in range(P):
        w[k, k // QUART] = 1.0
    return w.astype(ml_dtypes.bfloat16)


def _build_nc():
    import concourse.bass as bass
    import concourse.mybir as mybir
    from concourse.tile import TileContext

    nc = bass.Bass(trn_type="TRN2")
    x_i = nc.dram_tensor("x_i", [R, F], mybir.dt.float32, kind="ExternalInput")
    x_j = nc.dram_tensor("x_j", [R, F], mybir.dt.float32, kind="ExternalInput")
    ones_w = nc.dram_tensor("ones_w", [P, ROWS_PER_MEGA], mybir.dt.bfloat16,
                            kind="ExternalInput")
    cnt_out = nc.dram_tensor("cnt", [ROWS_PER_MEGA, 3 * M], mybir.dt.float32,
                             kind="ExternalOutput")

    xiv = x_i[:, :].flatten().rearrange("(m p f) -> m p f", p=P, f=W)
    xjv = x_j[:, :].flatten().rearrange("(m p f) -> m p f", p=P, f=W)

    gt = mybir.AluOpType.is_gt
    add = mybir.AluOpType.add
    f32 = mybir.dt.float32
    bf16 = mybir.dt.bfloat16

    with TileContext(nc) as tc:
        with tc.tile_pool(name="io", bufs=3) as iop, \
             tc.tile_pool(name="work", bufs=2) as wp, \
             tc.tile_pool(name="small", bufs=1) as sp, \
             tc.tile_pool(name="ps", bufs=2, space="PSUM") as pp:
            ones_t = sp.tile([P, ROWS_PER_MEGA], bf16)
            cnt = sp.tile([ROWS_PER_MEGA, 3 * M], f32)
            nc.sync.dma_start(out=ones_t, in_=ones_w[:, :])
            for m in range(LOOPS * M):
                lp, m = divmod(m, M)
                ti = iop.tile([P, W], f32, tag="ti")
                tj = iop.tile([P, W], f32, tag="tj")
                nc.sync.dma_start(out=ti, in_=xiv[m])
                nc.sync.dma_start(out=tj, in_=xjv[m])

                scr_s = wp.tile([P, W], f32, tag="scrs")
                q_i = wp.tile([P, W], bf16, tag="qi")
                q_s = wp.tile([P, W], bf16, tag="qs")
                sg = wp.tile([P, W], bf16, tag="sg")

                # GPSIMD does the fp32 add (frees the DVE); DVE does the
                # two quantize passes at 2x
                nc.gpsimd.tensor_tensor(scr_s[:, :], ti[:, :], tj[:, :], add)
                nc.vector.tensor_scalar(q_i[:, :], ti[:, :], 0.0, None, gt)
                nc.vector.tensor_scalar(q_s[:, :], scr_s[:, :], 0.0, None, gt)

                # ACT: strict (x_j > 0) as relu(sign(x_j))
                nc.scalar.activation(sg[:, :], tj[:, :],
                                     mybir.ActivationFunctionType.Sign)
                nc.scalar.activation(sg[:, :], sg[:, :],
                                     mybir.ActivationFunctionType.Relu)

                # PE: reduce partitions (4 quarters -> row), accumulate slices
                for t_idx, q in ((0, q_i), (1, sg), (2, q_s)):
                    ps = pp.tile([ROWS_PER_MEGA, 512], f32, tag=f"ps{t_idx}")
                    for s in range(NSLICE):
                        nc.tensor.matmul(
                            ps[:, :],
                            ones_t[:, :],
                            q[:, s * 512:(s + 1) * 512],
                            start=(s == 0),
                            stop=(s == NSLICE - 1),
                        )
                    nc.vector.tensor_reduce(
                        cnt[:, 3 * m + t_idx:3 * m + t_idx + 1],
                        ps[:, :],
                        axis=mybir.AxisListType.X,
                        op=add,
                    )
            nc.sync.dma_start(out=cnt_out[:, :], in_=cnt[:, :])
    return nc


def _split_multi_waits(nc):
    """Walrus in this toolchain encodes exactly one sync-wait per TPB
    instruction (NEURON_ISA_TPB_EVENTS has a single wait slot) and errors
    with "Too many sync wait commands" otherwise.  Tile freely attaches
    several waits to one instruction, so split them: hoist all but the last
    wait onto single-wait Drain carrier instructions inserted just before,
    on the same engine (sequential waits on one engine are equivalent)."""
    import copy as _copy

    import bass_rust
    import concourse.mybir as mb

    nidx = 0
    for f in nc.m.functions:
        new_blocks = []
        for blk in f.blocks:
            new_insts = []
            changed = False
            for ins in blk.instructions:
                si = ins.sync_info
                waits = list(si.on_wait) if si is not None and si.on_wait else []
                upds = list(si.on_update) if si is not None and si.on_update else []
                assert len(upds) <= 1, f"{ins.name}: {len(upds)} sync updates"
                if len(waits) > 1:
                    changed = True
                    for w in waits[:-1]:
                        nidx += 1
                        new_insts.append(mb.InstDrain(
                            name=f"waitsplit-{nidx}",
                            engine=ins.engine,
                            sync_info=bass_rust.SyncInfo(
                                on_wait=[w], on_update=[]),
                        ))
                    ins.sync_info = bass_rust.SyncInfo(
                        on_wait=[waits[-1]], on_update=upds)
                new_insts.append(ins)
            if changed:
                blk.set_instructions_from_list(new_insts) if hasattr(
                    blk, "set_instructions_from_list") else None
                if not hasattr(blk, "set_instructions_from_list"):
                    blk = _copy.replace(blk, instructions=new_insts)
            new_blocks.append(blk)
        if hasattr(f, "set_blocks_from_list"):
            f.set_blocks_from_list(new_blocks)
        else:
            f.blocks = new_blocks
    return nc


def _get_nc():
    if "nc" not in _CACHE:
        _CACHE["nc"] = _split_multi_waits(_build_nc())
    return _CACHE["nc"]


def _counts_from_cnt(cnt):
    """cnt: [32, 3*M] fp32 -> counts [3, R] (exact small ints as f32)."""
    A = cnt.reshape(ROWS_PER_MEGA, M, 3)               # (a, m, t)
    return A.transpose(2, 1, 0).reshape(3, M * ROWS_PER_MEGA)  # row = 32*m + a


def kernel(residue_i, residue_j):
    global LAST_RESULT
    from concourse.bass_utils import run_bass_kernel_spmd

    x_i = np.ascontiguousarray(np.asarray(residue_i, dtype=np.float32))
    x_j = np.ascontiguousarray(np.asarray(residue_j, dtype=np.float32))
    assert x_i.shape == (B, F) and x_j.shape == (B, F)

    nc = _get_nc()
    ones_np = _ones_block_np()
    in_maps = [
        {"x_i": x_i[c * R:(c + 1) * R], "x_j": x_j[c * R:(c + 1) * R],
         "ones_w": ones_np}
        for c in range(N_CORES)
    ]
    res = run_bass_kernel_spmd(nc, in_maps, core_ids=list(range(N_CORES)))
    LAST_RESULT = res

    counts = np.empty((3, B), dtype=np.float32)
    for c in range(N_CORES):
        counts[:, c * R:(c + 1) * R] = _counts_from_cnt(res.results[c]["cnt"])

    # --- entropy on host, float32 to mirror jnp ---
    n = np.float32(F)
    denom = n + np.float32(1e-8)
    c1 = counts.astype(np.float32)            # [3, B]: i, j, ij
    c0 = n - c1
    p0 = c0 / denom
    p1 = c1 / denom

    def term(p):
        return np.where(p > 0, p * np.log2(p + np.float32(1e-10)), np.float32(0.0))

    H = -(term(p0) + term(p1))                # [3, B]: H_i, H_j, H_ij
    E = (H[2] - H[0] - H[1]).astype(np.float32)
    is_co_prime = E >= np.float32(0.0)
    return (is_co_prime, E)
